# revision 1
# baseline (speedup 1.0000x reference)
"""Trainium2 Bass kernel for nn_CountingDiceLoss.

Key insight: in the reference, the cross-entropy term uses log_softmax over a
single-channel axis (identically zero) and a target clipped to index 0, so the
CE contribution is exactly 0 and the entire density-map computation (cent_i,
cent_j, bbox) is dead code.  The output reduces to the soft-dice loss over
classes 1 and 2:

    dc[b,c]  = (2*tp + s) / (sp + cnt + s),   s = 1e-5
    tp[b,c]  = sum_px softmax(x[b,:3])[c] * (y[b]==c)
    sp[b,c]  = sum_px softmax(x[b,:3])[c]
    cnt[b,c] = sum_px (y[b]==c)
    loss     = -mean_{b, c in {1,2}} dc[b,c]

Sharding: data-parallel over batch B=8, one sample per NeuronCore.  The host
packs each sample as one [128, 4, 8192] f32 plane (channels 0-2 = x logits,
channel 3 = y bit-cast to f32); each 2048-column chunk is loaded with two 2MB
DMAs (x0/x1 and x2/y) so consumers unblock as halves land.  Per chunk ACT
computes the three exps plus the reciprocal as exp(-ln(den)) - a manually
pre-loaded combined activation-table set (natural_log_exp_and_others) keeps
Exp and Ln resident together, so there is no per-chunk table-set switch
(which used to cost ~2.6us/chunk).  DVE computes den, the masks (is_equal on
the bit-cast y), and the four fp16 products, all in 2x perf mode; the
TensorEngine reduces all six quantities into one PSUM bank via
column-selector matmuls accumulated across chunks and passes.  Output per
core: 6 partial sums, combined on the host in float64.

The kernel is DMA-bound: 16MB of input per core per pass at ~330-430 GB/s
effective per-NeuronCore HBM bandwidth (~47-55us measured floor); ACT
(~38us), DVE (~36us) and PE (~29-37us) all sit below that, and every input
tile is consumed by cheap early ops so the DMA stream runs gapless.
"""

import os
import sys

import numpy as np

for _p in ("/opt/trn_rl_repo",):
    if _p not in sys.path and os.path.isdir(_p):
        sys.path.append(_p)

from contextlib import ExitStack

import concourse.bass as bass
import concourse.tile as tile
from concourse import bacc, mybir
from concourse.bass_utils import run_bass_kernel_spmd

P = 128          # SBUF partitions
WTOT = 8192      # free-dim length of one 1024x1024 plane laid out as [128, 8192]
FREE = int(os.environ.get("K_FREE", "2048"))  # chunk free size
MM = 512         # matmul free size (one PSUM bank of fp32)
NQ = 6           # reduced quantities: sp1, sp2, tp1, tp2, cnt1, cnt2
NBUF = int(os.environ.get("K_BUFS", "2"))
NBUF_IN = int(os.environ.get("K_BUFS_IN", "3"))
SMOOTH = 1e-5

f16 = mybir.dt.float16
f32 = mybir.dt.float32
i32 = mybir.dt.int32
AF = mybir.ActivationFunctionType
ALU = mybir.AluOpType


def _emit(ctx: ExitStack, tc: "tile.TileContext", out_ap, xy_ap, repeat=1,
          variant="full"):
    """xy_ap: [P, 4, WTOT] f32 dram - ch 0..2 = x logits, ch 3 = y (i32 bits).

    variant:
      full     - the production kernel: per chunk, two 2MB DMAs, three exps
                 (ACT), den + masks (DVE), then the exp(-ln(den)) reciprocal
                 on ACT (one pre-loaded combined table set - no table
                 switching), the prob/masked-prob products (DVE) and the PE
                 colsel reductions.
      pipe     - same ops, stage-2 emitted one chunk later (software
                 pipelining A/B probe; not reliably faster on HW)
      lnexp    - alias structure of full kept for A/B continuity
      dverecip - reciprocal on the DVE instead (A/B probe; ~20us/pass
                 slower on HW - InstReciprocal is an iterative divide)
      dmaonly  - loads + a token consume per chunk (DMA-floor probe)
    """
    nc = tc.nc

    if variant in ("full", "pipe", "lnexp"):
        # natural_log_exp_and_others: one ACT table set covering Exp and Ln.
        # Pre-loading it manually makes bacc's insert_act_table_loads pass a
        # no-op; otherwise it alternates exp_and_others / natural_log per
        # chunk (~1.3us per switch).
        nc.scalar.add_instruction(
            mybir.InstLoadActFuncSet(
                name=nc.get_next_instruction_name(),
                act_func_set_id=6,
            )
        )

    xin = ctx.enter_context(tc.tile_pool(name="xin", bufs=NBUF_IN))
    stg = ctx.enter_context(tc.tile_pool(name="stg", bufs=NBUF))
    loc = ctx.enter_context(tc.tile_pool(name="loc", bufs=int(os.environ.get("K_LOC", NBUF))))
    singles = ctx.enter_context(tc.tile_pool(name="singles", bufs=1))
    psum = ctx.enter_context(tc.tile_pool(name="psum", bufs=1, space="PSUM"))

    # column-selector stationary matrices: colsel[j] is [128, NQ] with ones in
    # column j.  matmul(acc, colsel[j], rhs) adds rhs's partition-sum into PSUM
    # row j and +0 into the other rows, so all six quantities share one bank.
    colsel = []
    for j in range(NQ):
        cs = singles.tile([P, NQ], f16, tag=f"colsel{j}")
        nc.vector.memset(cs, 0.0)
        nc.vector.memset(cs[:, j : j + 1], 1.0)
        colsel.append(cs)

    # one PSUM bank; row j accumulates quantity j as [1, MM] partials
    acc = psum.tile([NQ, MM], f32)

    plan = [(o, FREE) for o in range(0, WTOT, FREE)]
    total_chunks = repeat * len(plan)

    mm_count = [0]
    n_mm_total = total_chunks * NQ * ((FREE + MM - 1) // MM)

    def stage2(st):
        """Reciprocal + products + reductions for a stage-1 result."""
        csz = st["csz"]
        e1, e2, den, m1, m2, q1, q2 = (
            st["e1"], st["e2"], st["den"], st["m1"], st["m2"],
            st["q1"], st["q2"],
        )
        rr_t = loc.tile([P, FREE], f16, tag="rr")
        rr = rr_t[:, :csz]
        if variant == "dverecip":
            with nc.allow_low_precision(reason="fp16 softmax reciprocal"):
                nc.vector.reciprocal(rr, den)
        else:
            # exp(-ln(den)) on ACT; fp16 ln is plenty for the 2e-2 budget
            lg_t = loc.tile([P, FREE], f16, tag="lg")
            lg = lg_t[:, :csz]
            nc.scalar.activation(lg, den, AF.Ln)
            nc.scalar.activation(rr, lg, AF.Exp, scale=-1.0)

        p1_t = loc.tile([P, FREE], f16, tag="p1")
        p1 = p1_t[:, :csz]
        p2_t = loc.tile([P, FREE], f16, tag="p2")
        p2 = p2_t[:, :csz]
        nc.vector.tensor_mul(p1, e1, rr)
        nc.vector.tensor_mul(p2, e2, rr)
        nc.vector.tensor_mul(q1, p1, m1)
        nc.vector.tensor_mul(q2, p2, m2)

        for j, t in enumerate([p1, p2, q1, q2, m1, m2]):
            for s in range(0, csz, MM):
                n = min(MM, csz - s)
                nc.tensor.matmul(
                    acc[:, :n],
                    colsel[j],
                    t[:, s : s + n],
                    start=(mm_count[0] == 0),
                    stop=(mm_count[0] == n_mm_total - 1),
                )
                mm_count[0] += 1

    pending = None
    for rep, (k, (off, csz)) in (
        (r, c) for r in range(repeat) for c in enumerate(plan)
    ):
        sl = slice(off, off + csz)

        # two half-tiles: e0/e1 start after the first 2MB lands; e2 and the
        # masks wait only on the second
        xa_t = xin.tile([P, 2, FREE], f32, tag="xa")
        xb_t = xin.tile([P, 2, FREE], f32, tag="xb")
        nc.sync.dma_start(out=xa_t[:, :, :csz], in_=xy_ap[:, 0:2, sl])
        nc.sync.dma_start(out=xb_t[:, :, :csz], in_=xy_ap[:, 2:4, sl])
        x0 = xa_t[:, 0, :csz]
        x1 = xa_t[:, 1, :csz]
        x2 = xb_t[:, 0, :csz]
        yt = xb_t[:, 1, :csz].bitcast(i32)
        if variant == "dmaonly":
            # consume one column so DCE can't drop the loads
            junk = loc.tile([P, 2], f32, tag="junk")
            nc.vector.tensor_scalar(junk[:, 0:1], x0[:, 0:1], 0.0, None, ALU.add)
            nc.vector.tensor_scalar(junk[:, 1:2], x2[:, 0:1], 0.0, None, ALU.add)
            continue

        e0_t = stg.tile([P, FREE], f16, tag="e0")
        e0 = e0_t[:, :csz]
        e1_t = stg.tile([P, FREE], f16, tag="e1")
        e1 = e1_t[:, :csz]
        e2_t = stg.tile([P, FREE], f16, tag="e2")
        e2 = e2_t[:, :csz]
        nc.scalar.activation(e0, x0, AF.Exp)
        nc.scalar.activation(e1, x1, AF.Exp)
        nc.scalar.activation(e2, x2, AF.Exp)

        d01_t = stg.tile([P, FREE], f16, tag="d01")
        d01 = d01_t[:, :csz]
        den_t = stg.tile([P, FREE], f16, tag="den")
        den = den_t[:, :csz]
        nc.vector.tensor_add(d01, e0, e1)
        nc.vector.tensor_add(den, d01, e2)

        m1_t = stg.tile([P, FREE], f16, tag="m1")
        m1 = m1_t[:, :csz]
        m2_t = stg.tile([P, FREE], f16, tag="m2")
        m2 = m2_t[:, :csz]
        nc.vector.tensor_scalar(m1, yt, 1, None, ALU.is_equal)
        nc.vector.tensor_scalar(m2, yt, 2, None, ALU.is_equal)

        st = dict(csz=csz, e1=e1, e2=e2, den=den, m1=m1, m2=m2,
                  q1=e0, q2=d01)  # e0/d01 are dead after den; reuse for q1/q2

        if variant == "pipe":
            if pending is not None:
                stage2(pending)
            pending = st
        else:
            stage2(st)

    if pending is not None:
        stage2(pending)

    res = singles.tile([NQ, 1], f32)
    if variant == "dmaonly":
        nc.vector.memset(res, 0.0)
    else:
        nc.vector.reduce_sum(res, acc, axis=mybir.AxisListType.X)
    nc.sync.dma_start(out=out_ap, in_=res)


_NC_CACHE = {}


def _build_nc(repeat=1, variant="full"):
    key = (repeat, variant)
    if key not in _NC_CACHE:
        nc = bacc.Bacc(
            "TRN2",
            target_bir_lowering=False,
            debug=False,
            num_devices=8,
        )
        xy_ap = nc.dram_tensor("xc", [P, 4, WTOT], f32, kind="ExternalInput").ap()
        out_ap = nc.dram_tensor("out", [NQ, 1], f32, kind="ExternalOutput").ap()
        with tile.TileContext(nc) as tc:
            with ExitStack() as ctx:
                _emit(ctx, tc, out_ap, xy_ap, repeat=repeat, variant=variant)
        nc.compile()
        _NC_CACHE[key] = nc
    return _NC_CACHE[key]


def _get_nc():
    return _build_nc(1, os.environ.get("K_VARIANT", "full"))


def _pack_sample(xb: np.ndarray, yb: np.ndarray) -> np.ndarray:
    """[4,1024,1024] f32 logits + [1,1024,1024] i32 labels -> [128,4,8192]."""
    arr = np.empty((P, 4, WTOT), dtype=np.float32)
    for c in range(3):
        arr[:, c, :] = np.asarray(xb[c], dtype=np.float32).reshape(P, WTOT)
    arr[:, 3, :] = (
        np.ascontiguousarray(yb[0]).astype(np.int32).view(np.float32).reshape(P, WTOT)
    )
    return arr


def _run_cores(x: np.ndarray, y: np.ndarray, **spmd_kwargs):
    assert x.shape == (8, 4, 1024, 1024), x.shape
    assert y.shape == (8, 1, 1024, 1024), y.shape
    nc = _get_nc()
    in_maps = [{"xc": _pack_sample(x[b], y[b])} for b in range(8)]
    return run_bass_kernel_spmd(nc, in_maps, list(range(8)), **spmd_kwargs)


def _combine(results) -> np.float32:
    total = 0.0
    for b in range(8):
        o = np.asarray(results[b]["out"], dtype=np.float64).reshape(NQ)
        sp1, sp2, tp1, tp2, c1, c2 = o
        total += (2.0 * tp1 + SMOOTH) / (sp1 + c1 + SMOOTH)
        total += (2.0 * tp2 + SMOOTH) / (sp2 + c2 + SMOOTH)
    return np.float32(-total / 16.0)


def kernel(x, y, cent_i=None, cent_j=None, bbox=None) -> np.ndarray:
    # cent_i / cent_j / bbox only feed the density map, which is dead code in
    # the reference loss (CE term is identically zero).
    br = _run_cores(np.asarray(x), np.asarray(y))
    return _combine(br.results)

